# revision 1
# baseline (speedup 1.0000x reference)
"""BevFeatureEncoder on 8 Trainium2 NeuronCores.

Strategy (data-parallel over BEV grid slabs):
  - The 2*480*360 BEV cells are split into 8 contiguous ranges of 43200
    cells. Points are routed on host to the core owning their cell, so
    the segment_max reduction is fully local to each core.
  - On host (integer indexing only), each core's occupied cells are
    grouped by point count, counts padded up to k in {1,2,4,8,...} by
    duplicating points of the same cell (a no-op under max). Cells are
    processed in chunks of 512/256; points are laid out so slot s of a
    chunk is a dense, contiguous block of points. The on-device
    segment_max is then dense max ops per chunk - no gather/scatter.
  - Device dataflow per slot block: 3-layer MLP as matmuls with BN scale
    folded into the weights (diag(s) matmul + PE transposes at startup),
    so each BN+ReLU is a single bias+relu op. Layer-1 packs TWO cells
    per column (block-diagonal stationary [8, 128], K=8): the mm1 and
    its bias+relu run at half the columns; layer 2 unpacks via
    even/odd-half weight matrices ([128,128] with zero halves).
    Layers 2/3 and the compression run in bf16 (weights + activations),
    which keeps the PE at 1 col/cycle at any width and halves
    SBUF/weight-load traffic. Max runs over raw layer-3 outputs (folded
    scale > 0 commutes with max), bias+relu once per chunk, then the
    compression matmul and one DMA of the [64, c] result per chunk.
  - The bias+relu ops are debt-balanced across ACT and DVE (the only
    engines with PSUM access); bias+max accumulation runs on DVE. The
    Pool engine issues the output DMAs and startup weight loads; input
    vox prefetch is issued from the sync engine.
  - Device output is compacted chunk-major [64, total_cells]; the host
    unshard places real columns into the zeroed [B, C, GX, GY] grid.
  - Chunk structure is equalized across cores (padded with dummy cells,
    dropped at unshard) so a single SPMD program serves all 8 cores.
"""

import numpy as np

import concourse.bacc as bacc
import concourse.bass as bass
import concourse.mybir as mybir
import concourse.tile as tile
from concourse import bass_utils
from concourse.masks import make_identity

GX, GY = 480, 360
B = 2
EPS = 1e-5
N_CORES = 8
CELLS_PER_CORE = (B * GX * GY) // N_CORES  # 43200
CHUNK = 512  # cells per chunk
PAD = -1  # pad-cell marker in the row table
VOX_GRP = 2048  # vox DMA group width (columns)

F32 = mybir.dt.float32
F32R = mybir.dt.float32r
BF16 = mybir.dt.bfloat16

Relu = mybir.ActivationFunctionType.Relu


# ---------------------------------------------------------------- host prep


def _prep_core(seg_local, lo_idx):
    """Group one core's occupied cells by padded point count.

    Returns {k: (cells [n], slots [n, k])}, point indices into the global
    voxel array, slots padded by repeating the last point of the cell.
    """
    order = np.argsort(seg_local, kind="stable")
    seg_sorted = seg_local[order]
    cells, starts, counts = np.unique(
        seg_sorted, return_index=True, return_counts=True
    )
    ks2 = 1 << (np.ceil(np.log2(np.maximum(counts, 1))).astype(np.int64))
    ks = np.where(counts <= 4, counts, np.maximum(ks2, 1)).astype(np.int64)
    out = {}
    for k in np.unique(ks):
        sel = np.nonzero(ks == k)[0]
        slots = np.empty((len(sel), int(k)), np.int64)
        for s in range(int(k)):
            slots[:, s] = order[starts[sel] + np.minimum(s, counts[sel] - 1)]
        out[int(k)] = (cells[sel].astype(np.int64), lo_idx[slots])
    return out


def _layout_items(chunk_plan):
    """Flat slot-item stream (ci, k, c, slot) and vox DMA group layout.

    Vox columns are cell-PAIR packed (two cells per column: chunk cells
    j and c/2+j in partition rows 0:4 / 4:8), so an item occupies c/2
    vox columns. Pairs of consecutive items share PSUM tiles; pairs are
    packed whole into vox groups of <= VOX_GRP packed columns, each
    group filling one ring buffer via one DMA.
    Returns (items, pairs, item_src, ngroups); item_src maps item ->
    (group idx, packed-col offset inside group).
    """
    items = []
    for ci, (k, c) in enumerate(chunk_plan):
        for s in range(k):
            items.append((ci, k, c, s))

    pairs = []
    i = 0
    while i < len(items):
        if i + 1 < len(items):
            pairs.append([items[i], items[i + 1]])
            i += 2
        else:
            pairs.append([items[i]])
            i += 1

    # pack pairs (contiguously) into vox groups of packed columns
    item_src = {}  # (ci,k,c,s) -> (group, packed offset)
    ngroups = 0
    goff = VOX_GRP  # force new group at start
    for pr in pairs:
        w = sum(it[2] // 2 for it in pr)
        if goff + w > VOX_GRP:
            g = ngroups
            ngroups += 1
            goff = 0
        else:
            g = ngroups - 1
        for it in pr:
            item_src[it[:4]] = (g, goff)
            goff += it[2] // 2
    return items, pairs, item_src, ngroups


def _build_plan_and_data(voxels, coors):
    """Route points to cores, build the equalized chunk plan plus per-core
    device inputs (vox laid out [4, vox_cols]) and host placement tables.

    chunk_plan: list of (k, c) with c in {256, 512}.
    """
    seg = (
        coors[:, 0].astype(np.int64) * (GX * GY)
        + coors[:, 1].astype(np.int64) * GY
        + coors[:, 2].astype(np.int64)
    )
    core_of = seg // CELLS_PER_CORE
    per_core = []
    for c in range(N_CORES):
        idx = np.nonzero(core_of == c)[0]
        per_core.append(_prep_core(seg[idx] - c * CELLS_PER_CORE, idx))

    # high-k chunks first: their serial max chains overlap the busy
    # middle of the run instead of serializing the drain tail
    all_ks = sorted({k for g in per_core for k in g.keys()}, reverse=True)
    chunk_plan = []  # (k, c)
    for k in all_ks:
        n_max = max(len(g[k][0]) if k in g else 0 for g in per_core)
        n_pad = -(-n_max // 256) * 256  # c multiples of 256 only
        while n_pad > 0:
            c = min(n_pad, CHUNK)
            if c == 384:
                c = 256
            chunk_plan.append((k, c))
            n_pad -= c
    total_cells = sum(c for _, c in chunk_plan)

    items, pairs, item_src, ngroups = _layout_items(chunk_plan)
    vox_cols = ngroups * VOX_GRP

    vox_all = np.zeros((N_CORES, 8, vox_cols), np.float32)
    rows_all = np.full((N_CORES, total_cells), PAD, np.int64)

    for core in range(N_CORES):
        groups = per_core[core]
        cell0 = 0
        used = {}
        chunk_slots = {}  # ci -> [c, k] point indices
        for ci, (k, c) in enumerate(chunk_plan):
            cells, slots = groups.get(
                k, (np.zeros(0, np.int64), np.zeros((0, k), np.int64)))
            u = used.get(k, 0)
            batch_cells = cells[u : u + c]
            batch_slots = slots[u : u + c]
            used[k] = u + c
            nb = len(batch_cells)
            sl = np.zeros((c, k), np.int64)
            if nb:
                sl[:nb] = batch_slots
                sl[nb:] = batch_slots[0, 0]
            elif len(cells):
                sl[:] = slots[0, 0]
            chunk_slots[ci] = sl
            rows_all[core, cell0 : cell0 + nb] = batch_cells
            cell0 += c
        for (ci, k, c, s), (g, off) in item_src.items():
            col = g * VOX_GRP + off
            h = c // 2
            sl = chunk_slots[ci]
            vox_all[core, 0:4, col : col + h] = voxels[sl[:h, s]].T
            vox_all[core, 4:8, col : col + h] = voxels[sl[h:, s]].T
    return chunk_plan, vox_cols, vox_all, rows_all


# ------------------------------------------------------------- bass program


def build_program(chunk_plan, vox_cols):
    total_cells = sum(c for _, c in chunk_plan)
    items, pairs, item_src, ngroups = _layout_items(chunk_plan)
    nc = bacc.Bacc("TRN2", target_bir_lowering=False, debug=False,
                   num_devices=N_CORES)

    vox = nc.dram_tensor("vox", [8, vox_cols], F32R,
                         kind="ExternalInput").ap()
    w_in = {}
    for name, shape in [
        ("w1", [4, 64]), ("w2", [64, 128]), ("w3", [128, 256]),
        ("wc", [256, 64]), ("bc", [64]),
        ("g1", [64]), ("b1", [64]), ("m1", [64]), ("v1", [64]),
        ("g2", [128]), ("b2", [128]), ("m2", [128]), ("v2", [128]),
        ("g3", [256]), ("b3", [256]), ("m3", [256]), ("v3", [256]),
    ]:
        w_in[name] = nc.dram_tensor(name, shape, F32, kind="ExternalInput").ap()
    comp = nc.dram_tensor("comp", [64, total_cells], F32,
                          kind="ExternalOutput").ap()

    from contextlib import ExitStack
    with tile.TileContext(nc) as tc, ExitStack() as ctx:
        cpool = ctx.enter_context(tc.tile_pool(name="const", bufs=1))

        ident = cpool.tile([128, 128], F32)
        make_identity(nc, ident[:])
        eps_tile = cpool.tile([128, 1], F32)
        nc.vector.memset(eps_tile[:], EPS)

        # bn vectors are loaded as [1, c] rows (single-descriptor DMAs,
        # spread across engine queues) and flipped to [c, 1] columns with
        # one PE transpose per layer; startup weight loads go on the Pool
        # queue so the sync queue is free to prefetch vox immediately.
        _bnq = [nc.gpsimd, nc.scalar, nc.scalar, nc.gpsimd]

        with tc.tile_pool(name="fold_ps", bufs=2, space="PSUM") as fps, \
             tc.tile_pool(name="fold_sb", bufs=2) as fsb:

            def bn_scale_bias(li, c, half=None):
                sfx = f"{li}_{half}"
                rows = fsb.tile([4, c], F32, tag="bn_rows")
                for i, pfx in enumerate("gbmv"):
                    src = w_in[f"{pfx}{li}"]
                    if half is not None:
                        src = src[half * 128 : half * 128 + c]
                    _bnq[i].dma_start(out=rows[i : i + 1, :], in_=src[None, :])
                pT = fps.tile([c, 4], F32, tag="bn_pT", space="PSUM")
                nc.tensor.transpose(out=pT[:], in_=rows[:],
                                    identity=ident[:4, :4])
                cols = cpool.tile([c, 4], F32, tag=f"bn_cols{sfx}")
                nc.vector.tensor_copy(cols[:], pT[:])
                g, b, m, v = (cols[:, i : i + 1] for i in range(4))
                sq = cpool.tile([c, 1], F32, tag=f"bn_sq{sfx}")
                nc.scalar.activation(sq[:], v,
                                     mybir.ActivationFunctionType.Sqrt,
                                     bias=eps_tile[:c, :])
                s = cpool.tile([c, 1], F32, tag=f"bn_s{sfx}")
                nc.vector.reciprocal(s[:], sq[:])
                nc.vector.tensor_mul(s[:], g, s[:])
                t = cpool.tile([c, 1], F32, tag=f"bn_t{sfx}")
                nc.vector.tensor_mul(t[:], m, s[:])
                nc.vector.tensor_sub(t[:], b, t[:])
                return s, t

            def fold(name, kin, cout, w_src, s_ap, out_dt=BF16):
                wr = fsb.tile([kin, cout], F32, tag="fold_raw")
                nc.gpsimd.dma_start(out=wr[:], in_=w_src)
                pT = fps.tile([cout, kin], F32, tag="fold_pT", space="PSUM")
                nc.tensor.transpose(out=pT[:], in_=wr[:],
                                    identity=ident[:kin, :kin])
                wT = fsb.tile([cout, kin], F32, tag="fold_wT")
                nc.vector.tensor_copy(wT[:], pT[:])
                dg = fsb.tile([cout, cout], F32, tag="fold_dg")
                nc.vector.tensor_scalar_mul(dg[:], ident[:cout, :cout], s_ap)
                pS = fps.tile([cout, kin], F32, tag="fold_pS", space="PSUM")
                nc.tensor.matmul(pS[:], dg[:], wT[:], start=True, stop=True)
                wsT = fsb.tile([cout, kin], F32, tag="fold_wsT")
                nc.vector.tensor_copy(wsT[:], pS[:])
                pB = fps.tile([kin, cout], F32, tag="fold_pB", space="PSUM")
                nc.tensor.transpose(out=pB[:], in_=wsT[:],
                                    identity=ident[:cout, :cout])
                out = cpool.tile([kin, cout], out_dt, tag=name)
                nc.vector.tensor_copy(out[:], pB[:])
                return out

            # layer 1 chain first (gates the hot loop's first mm1)
            s1, t1 = bn_scale_bias(1, 64)
            w1s4 = fold("w1s4", 4, 64, w_in["w1"], s1[:], out_dt=F32R)
            # layer-1 block-diagonal stationary [8, 128]: two cells/column
            w1d8 = cpool.tile([8, 128], F32R)
            nc.vector.memset(w1d8[:].bitcast(mybir.dt.uint32), 0)
            nc.vector.tensor_copy(w1d8[0:4, 0:64], w1s4[:])
            # partition-shifted copy goes via SBUF->SBUF DMA
            nc.gpsimd.dma_start(out=w1d8[4:8, 64:128], in_=w1s4[:])
            # stacked layer-1 bias [128, 1]
            t1d2 = cpool.tile([128, 1], F32)
            nc.vector.tensor_copy(t1d2[0:64, :], t1[:])
            nc.gpsimd.dma_start(out=t1d2[64:128, :], in_=t1[:])

            # layer 2
            s2, t2 = bn_scale_bias(2, 128)
            w2s = fold("w2s", 64, 128, w_in["w2"], s2[:])
            # layer-2 even/odd stationaries [128, 128] (zero half each)
            w2e = cpool.tile([128, 128], BF16)
            nc.vector.memset(w2e[:], 0.0)
            nc.vector.tensor_copy(w2e[0:64, :], w2s[:])
            w2o = cpool.tile([128, 128], BF16)
            nc.vector.memset(w2o[:], 0.0)
            nc.gpsimd.dma_start(out=w2o[64:128, :], in_=w2s[:])

            # layer 3
            s3a, t3a = bn_scale_bias(3, 128, half=0)
            w3a = fold("w3a", 128, 128, w_in["w3"][:, 0:128], s3a[:])
            s3b, t3b = bn_scale_bias(3, 128, half=1)
            w3b = fold("w3b", 128, 128, w_in["w3"][:, 128:256], s3b[:])

        def load_cast(name, shape, src_ap):
            raw = cpool.tile(shape, F32, tag=name + "_raw")
            nc.gpsimd.dma_start(out=raw[:], in_=src_ap)
            rnd = cpool.tile(shape, BF16, tag=name)
            nc.vector.tensor_copy(rnd[:], raw[:])
            return rnd

        wc0 = load_cast("wc0", [128, 64], w_in["wc"][0:128, :])
        wc1 = load_cast("wc1", [128, 64], w_in["wc"][128:256, :])
        # compression bias stacked for paired-chunk finalize
        bc2 = cpool.tile([128, 1], F32)
        nc.scalar.dma_start(out=bc2[0:64, :], in_=w_in["bc"][:, None])
        nc.scalar.dma_start(out=bc2[64:128, :], in_=w_in["bc"][:, None])
        bc = bc2[0:64, :]

        # vox ring: persistent [4, VOX_GRP] buffers, one DMA per group
        N_VOXBUF = 4
        voxbuf = []
        for i in range(N_VOXBUF):
            vb = cpool.tile([8, VOX_GRP], F32R, tag=f"voxbuf{i}")
            voxbuf.append(vb)

        sb = ctx.enter_context(tc.tile_pool(name="sb", bufs=8))
        scp = ctx.enter_context(tc.tile_pool(name="scp", bufs=4))
        # PSUM (8 banks): p12 ring 2x[128,1024] (4 banks, wide pair tiles),
        # ps3 ring 4x[128,512] (4 banks, also hosts pc tiles)
        p12 = ctx.enter_context(tc.tile_pool(name="p12", bufs=2, space="PSUM"))
        ps3 = ctx.enter_context(tc.tile_pool(name="ps3", bufs=4, space="PSUM"))

        # debt-balanced routing of bias+relu ops over ACT / DVE (the only
        # engines with PSUM access); bias+max accumulation is DVE-only
        debt = {"act": 0.0, "dve": 0.0}
        COST = {
            "act": lambda c: (c + 190.0) / 1.2,
            "dve": lambda c: (c + 120.0) / 0.96,
        }

        def br_auto(out_ap, in_ap, bias_ap):
            """out = relu(in + bias) on the least-loaded engine."""
            c = in_ap.shape[-1]
            eng = min(("act", "dve"), key=lambda e: debt[e] + COST[e](c))
            debt[eng] += COST[eng](c)
            if eng == "act":
                nc.scalar.activation(out_ap, in_ap, Relu, bias=bias_ap,
                                     scale=1.0)
            else:
                nc.vector.tensor_scalar(out_ap, in_ap, bias_ap, 0.0,
                                        op0=mybir.AluOpType.add,
                                        op1=mybir.AluOpType.max)

        def max_br_auto(out_ap, in_ap, bias_ap, acc_ap):
            """out = max(in + bias, acc) on DVE."""
            c = in_ap.shape[-1]
            debt["dve"] += COST["dve"](c)
            nc.vector.scalar_tensor_tensor(out_ap, in_ap, bias_ap, acc_ap,
                                           op0=mybir.AluOpType.add,
                                           op1=mybir.AluOpType.max)

        cell_off = []
        co = 0
        for k, c in chunk_plan:
            cell_off.append(co)
            co += c

        # chunk state
        accA = {}
        accB = {}

        def finalize_chunks(group):
            """Compression + bias-relu + output DMA for 1 or 2 chunks.

            Two adjacent equal-width chunks share one [128, c] PSUM tile
            (chunk A in partitions 0:64, chunk B in 64:128), one act op,
            and one rearranged output DMA.
            """
            (ci, k, c) = group[0]
            pc = ps3.tile([128, c], F32, tag="p3", space="PSUM",
                          name=f"pc{ci}")
            for row, (cj, kj, cjw) in enumerate(group):
                a_ap, b_ap = accA.pop(cj), accB.pop(cj)
                dst = pc[64 * row : 64 * row + 64, :]
                nc.tensor.matmul(dst, wc0[:], a_ap, start=True, stop=False)
                nc.tensor.matmul(dst, wc1[:], b_ap, start=False, stop=True)
            n = len(group)
            sc = scp.tile([64 * n, c], F32, tag="sc", name=f"sc{ci}")
            br_auto(sc[:], pc[0 : 64 * n, :], bc2[0 : 64 * n, :])
            o = cell_off[ci]
            nc.gpsimd.dma_start(out=comp[:, o : o + c], in_=sc[0:64, :])
            if n == 2:
                nc.gpsimd.dma_start(out=comp[:, o + c : o + 2 * c],
                                    in_=sc[64:128, :])

        def stage1(pair):
            """mm1 (K=8, cell-pair packed) -> h1 for a pair of items.

            The pair's packed vox blocks are contiguous in one ring
            buffer, so a single matmul covers the whole pair.
            """
            Wp = sum(it[2] // 2 for it in pair)
            nm = f"{pair[0][0]}_{pair[0][3]}"
            p1f = p12.tile([128, 1024], F32, tag="p12", space="PSUM",
                           name=f"p1_{nm}")
            p1 = p1f[:, 0:Wp]
            g, goff = item_src[pair[0][:4]]
            vb = voxbuf[g % N_VOXBUF]
            nc.tensor.matmul(p1[:], w1d8[:], vb[:, goff : goff + Wp],
                             start=True, stop=True)
            h1 = sb.tile([128, Wp], BF16, tag="h1", name=f"h1_{nm}")
            br_auto(h1[:], p1[:], t1d2[:])
            return h1

        def stage2(pair, h1):
            """mm2 (K=128 even/odd halves, bf16) -> h2 for a stage1 pair.

            Per item: two matmuls (first/second-half weights) write the
            item's contiguous [first-half | second-half] block of p2, so
            h2 stays contiguous per item for the mm3 moving operand.
            """
            W = sum(it[2] for it in pair)
            nm = f"{pair[0][0]}_{pair[0][3]}"
            p2f = p12.tile([128, 1024], F32, tag="p12", space="PSUM",
                           name=f"p2_{nm}")
            p2 = p2f[:, 0:W]
            o = 0
            ho = 0
            for it in pair:
                c = it[2]
                h = c // 2
                nc.tensor.matmul(p2[:, o : o + h], w2e[:],
                                 h1[:, ho : ho + h], start=True, stop=True)
                nc.tensor.matmul(p2[:, o + h : o + c], w2o[:],
                                 h1[:, ho : ho + h], start=True, stop=True)
                o += c
                ho += h
            h2 = sb.tile([128, W], BF16, tag="h2", name=f"h2_{nm}")
            br_auto(h2[:], p2[:], t2[:])
            return h2

        fin_q = []

        def stage3(it, h2_ap):
            """mm3a/b (bf16) -> running affine-relu'd max; queue finalize."""
            ci, k, c, s = it
            p3A = ps3.tile([128, c], F32, tag="p3", space="PSUM",
                           name=f"p3A_{ci}_{s}")
            p3B = ps3.tile([128, c], F32, tag="p3", space="PSUM",
                           name=f"p3B_{ci}_{s}")
            nc.tensor.matmul(p3A[:], w3a[:], h2_ap, start=True, stop=True)
            nc.tensor.matmul(p3B[:], w3b[:], h2_ap, start=True, stop=True)
            if k == 1:
                aA = sb.tile([128, c], BF16, tag="accrA", name=f"aA_{ci}")
                br_auto(aA[:], p3A[:], t3a[:])
                aB = sb.tile([128, c], BF16, tag="accrB", name=f"aB_{ci}")
                br_auto(aB[:], p3B[:], t3b[:])
                accA[ci], accB[ci] = aA[:], aB[:]
            elif s == 0:
                aA = [sb.tile([128, c], BF16, tag="accrA",
                              name=f"aA_{ci}_{j}") for j in range(2)]
                aB = [sb.tile([128, c], BF16, tag="accrB",
                              name=f"aB_{ci}_{j}") for j in range(2)]
                br_auto(aA[0][:], p3A[:], t3a[:])
                br_auto(aB[0][:], p3B[:], t3b[:])
                accA[ci] = aA
                accB[ci] = aB
                accA[f"n{ci}"] = 0
                accB[f"n{ci}"] = 0
            else:
                nA = accA[f"n{ci}"]
                max_br_auto(accA[ci][1 - nA][:], p3A[:], t3a[:],
                            accA[ci][nA][:])
                accA[f"n{ci}"] = 1 - nA
                nB = accB[f"n{ci}"]
                max_br_auto(accB[ci][1 - nB][:], p3B[:], t3b[:],
                            accB[ci][nB][:])
                accB[f"n{ci}"] = 1 - nB
            if s == k - 1:
                if k > 1:
                    nA, nB = accA.pop(f"n{ci}"), accB.pop(f"n{ci}")
                    accA[ci] = accA[ci][nA][:]
                    accB[ci] = accB[ci][nB][:]
                fin_q.append((ci, k, c))

        # software-pipelined emission over item pairs with vox prefetch
        h1q = []
        h2q = []
        n_pairs = len(pairs)
        PREFETCH = 3  # groups of vox DMA ahead of consumption
        next_grp = 0

        def issue_vox(upto):
            nonlocal next_grp
            while next_grp < min(upto, ngroups):
                g = next_grp
                vb = voxbuf[g % N_VOXBUF]
                nc.sync.dma_start(
                    out=vb[:], in_=vox[:, g * VOX_GRP : (g + 1) * VOX_GRP])
                next_grp += 1

        issue_vox(PREFETCH)
        for t in range(n_pairs + 5):
            ready_fins = list(fin_q)
            fin_q.clear()
            if t < n_pairs:
                # keep the vox ring PREFETCH groups ahead of this pair
                g_now = item_src[pairs[t][0][:4]][0]
                issue_vox(g_now + PREFETCH)
                h1q.append((pairs[t], stage1(pairs[t])))
            if t >= 2 and h1q:
                pr, h1 = h1q.pop(0)
                h2q.append((pr, stage2(pr, h1)))
            if t >= 4 and h2q:
                pr, h2 = h2q.pop(0)
                o = 0
                for it in pr:
                    stage3(it, h2[:, o : o + it[2]])
                    o += it[2]
            i = 0
            while i < len(ready_fins):
                f = ready_fins[i]
                if (i + 1 < len(ready_fins)
                        and ready_fins[i + 1][0] == f[0] + 1
                        and ready_fins[i + 1][2] == f[2]):
                    finalize_chunks([f, ready_fins[i + 1]])
                    i += 2
                else:
                    finalize_chunks([f])
                    i += 1

    nc.compile()
    return nc


# ------------------------------------------------------------------ driver

_CACHE = {}


def kernel(voxels, coors, batch_size, w1, g1, b1, m1, v1,
           w2, g2, b2, m2, v2, w3, g3, b3, m3, v3, wc, bc,
           _trace=False):
    voxels = np.asarray(voxels, np.float32)
    coors = np.asarray(coors, np.int32)
    chunk_plan, vox_cols, vox_all, rows_all = _build_plan_and_data(
        voxels, coors)

    key = tuple(chunk_plan)
    if key not in _CACHE:
        _CACHE[key] = build_program(chunk_plan, vox_cols)
    nc = _CACHE[key]

    weights = {
        k: np.asarray(v, np.float32)
        for k, v in [
            ("w1", w1), ("w2", w2), ("w3", w3), ("wc", wc), ("bc", bc),
            ("g1", g1), ("b1", b1), ("m1", m1), ("v1", v1),
            ("g2", g2), ("b2", b2), ("m2", m2), ("v2", v2),
            ("g3", g3), ("b3", b3), ("m3", m3), ("v3", v3),
        ]
    }
    in_maps = [{"vox": vox_all[c], **weights} for c in range(N_CORES)]
    res = bass_utils.run_bass_kernel_spmd(
        nc, in_maps, core_ids=list(range(N_CORES)), trace=_trace)

    # unshard: place compacted columns into the zeroed channel-major grid
    out = np.zeros((B, 64, GX * GY), np.float32)
    for c in range(N_CORES):
        cm = res.results[c]["comp"]  # [64, total_cells]
        rows = rows_all[c]  # [total_cells] local slab rows, PAD for dummy
        real = rows != PAD
        gcell = rows[real] + c * CELLS_PER_CORE
        b_core = c // (N_CORES // B)
        xy = gcell - b_core * (GX * GY)
        out[b_core][:, xy] = cm[:, real]
    out = out.reshape(B, 64, GX, GY)
    if _trace:
        return out, res
    return out

